# revision 1
# baseline (speedup 1.0000x reference)
"""Trainium2 Bass kernel for a CRF loss (log-likelihood) over B=128, S=1024, T=128.

Strategy
--------
log-denominator (the forward algorithm) is a chain of 1023 dependent
(matmul, elementwise-multiply) rounds in the exp domain:

    a_s = (E^T_lhsT a_{s-1}) * exp(x_s)        E = exp(transitions)

The chain is split in half: cores 0-3 run the forward recurrence for
s=0..511 and cores 4-7 run the *backward* recurrence for s=1023..512
(same program shape: the backward chain is the forward chain applied to the
sequence-reversed inputs with the transposed transition matrix), halving the
serial-latency wall.  Batch is sharded 4 ways (32 per core) over each
direction.  The two half-chain states are combined on the host via
    log_den[b] = log(sum_j u_fwd[j,b] * g_bwd[j,b]) + sum(log rescales)
where u_fwd is one extra matmul-only round on the forward side.

Periodic rescaling (every 8 rounds, applied lazily 3 rounds later so the
reciprocal pipeline stays off the critical path) keeps values inside fp32
range; the norms are written out and their logs accumulated on the host in
float64.

log-numerator (gathers) is computed on-device with one-hot tricks: emissions
via iota==tag one-hot (tensor_scalar per-partition compare) then multiply +
reduce; transition scores via a pair-count matrix accumulated with PE matmuls
of shifted one-hots, dotted with the transition matrix; start/end lookups via
a 32-row indirect DMA.  Each core handles 16 batch rows of numerator work.

NOTE on op choices (all HW-bisected on the axon trn2 path): tensor_tensor with
a step-0 to_broadcast operand and tensor_tensor_reduce both crash the device;
tensor_scalar with an AP scalar works.  The program must be built with
bacc.Bacc + nc.compile() — raw bass.Bass fails walrus codegen on multi-sem
waits.

The final scalar is assembled on the host (the all-reduce over 8 partial
results).
"""

import os
import sys

for _p in ("/opt/trn_rl_repo", "/root/.axon_site/_ro/trn_rl_repo"):
    if os.path.isdir(_p) and _p not in sys.path:
        sys.path.append(_p)

from contextlib import ExitStack

import numpy as np

import concourse.bass as bass
import concourse.tile as tile
from concourse import bacc, mybir
from concourse import bass_utils

B, S, T = 128, 1024, 128
R = 512          # rounds per direction (s* = 511)
BL = 32          # batch per recurrence core
NB = 16          # batch per numerator shard
KN = 8           # rescale interval (rounds)
DELAY = 3        # rescale application delay (rounds)
RESCALE_ROUNDS = [r for r in range(KN, R, KN) if r + DELAY <= R - 1]
NRESC = len(RESCALE_ROUNDS)

F32 = mybir.dt.float32
I32 = mybir.dt.int32
BF16 = mybir.dt.bfloat16
AF = mybir.ActivationFunctionType
OP = mybir.AluOpType


def build_program(rounds=R, numerator=True, se=True, rescale=True, num_mode="full"):
    nc = bacc.Bacc(
        "TRN2",
        target_bir_lowering=False,
        debug=False,
        enable_asserts=False,
        num_devices=8,
    )

    # ---- DRAM I/O ----------------------------------------------------------
    xt_d = nc.dram_tensor("xt", (T, R * BL), F32, kind="ExternalInput")
    wraw_d = nc.dram_tensor("wraw", (T, T), F32, kind="ExternalInput")
    bias_d = nc.dram_tensor("bias", (T, 1), F32, kind="ExternalInput")
    trans_d = nc.dram_tensor("transfull", (T, T), F32, kind="ExternalInput")
    tags_d = nc.dram_tensor("tags_sT", (128, 128), F32, kind="ExternalInput")
    tagsn_d = nc.dram_tensor("tagsn_sT", (128, 128), F32, kind="ExternalInput")
    setags_d = nc.dram_tensor("setags", (2 * NB, 1), I32, kind="ExternalInput")
    setable_d = nc.dram_tensor("setable", (2 * T, 1), F32, kind="ExternalInput")
    iota_d = nc.dram_tensor("iota_t", (128, T), F32, kind="ExternalInput")
    onescol_d = nc.dram_tensor("ones_col", (128, 1), F32, kind="ExternalInput")
    onesrow_d = nc.dram_tensor("ones_row", (1, 128), F32, kind="ExternalInput")
    xnat_d = nc.dram_tensor("xnat", (NB, S, T), F32, kind="ExternalInput")

    out_u_d = nc.dram_tensor("out_u", (T, BL), F32, kind="ExternalOutput")
    out_a_d = nc.dram_tensor("out_a", (T, BL), F32, kind="ExternalOutput")
    out_nrms_d = nc.dram_tensor("out_nrms", (1, NRESC * BL), F32, kind="ExternalOutput")
    out_num_d = nc.dram_tensor("out_num", (1, NB * (S // 128 + 1)), F32, kind="ExternalOutput")
    out_se_d = nc.dram_tensor("out_se", (2 * NB, 1), F32, kind="ExternalOutput")

    with ExitStack() as ctx:
        tc = ctx.enter_context(tile.TileContext(nc))
        singles = ctx.enter_context(tc.tile_pool(name="singles", bufs=1))
        raw = ctx.enter_context(tc.tile_pool(name="raw", bufs=3))
        apool = ctx.enter_context(tc.tile_pool(name="apool", bufs=4))
        rpool = ctx.enter_context(tc.tile_pool(name="rpool", bufs=2))
        xn_pool = ctx.enter_context(tc.tile_pool(name="xn_pool", bufs=4))
        oh_pool = ctx.enter_context(tc.tile_pool(name="oh_pool", bufs=4))
        scr_pool = ctx.enter_context(tc.tile_pool(name="scr_pool", bufs=2))
        psum_p = ctx.enter_context(tc.tile_pool(name="psum_p", bufs=2, space="PSUM"))
        psum_b = ctx.enter_context(tc.tile_pool(name="psum_b", bufs=2, space="PSUM"))
        psum_n = ctx.enter_context(tc.tile_pool(name="psum_n", bufs=1, space="PSUM"))
        psum_c = ctx.enter_context(tc.tile_pool(name="psum_c", bufs=1, space="PSUM"))

        # ---- constants -----------------------------------------------------
        w_raw = singles.tile([T, T], F32)
        nc.sync.dma_start(out=w_raw, in_=wraw_d.ap())
        w_sb = singles.tile([T, T], F32)
        nc.scalar.activation(w_sb, w_raw, AF.Exp)

        bias_sb = singles.tile([T, 1], F32)
        nc.sync.dma_start(out=bias_sb, in_=bias_d.ap())
        trans_sb = singles.tile([T, T], F32)
        nc.sync.dma_start(out=trans_sb, in_=trans_d.ap())
        iota_sb = singles.tile([128, T], F32)
        nc.sync.dma_start(out=iota_sb, in_=iota_d.ap())
        onescol_sb = singles.tile([128, 1], F32)
        nc.sync.dma_start(out=onescol_sb, in_=onescol_d.ap())
        onesrow_sb = singles.tile([1, 128], F32)
        nc.sync.dma_start(out=onesrow_sb, in_=onesrow_d.ap())
        tags_sb = singles.tile([128, 128], F32)
        nc.sync.dma_start(out=tags_sb, in_=tags_d.ap())
        tagsn_sb = singles.tile([128, 128], F32)
        nc.sync.dma_start(out=tagsn_sb, in_=tagsn_d.ap())
        setags_sb = singles.tile([2 * NB, 1], I32)
        nc.sync.dma_start(out=setags_sb, in_=setags_d.ap())

        # ---- chain init first (keeps ACT from serializing the chain start
        # behind the bulk exp staging) -----------------------------------
        initraw = singles.tile([T, BL], F32)
        nc.sync.dma_start(out=initraw, in_=xt_d.ap()[:, 0:BL])
        a_prev = apool.tile([T, BL], F32, name="a")
        nc.scalar.activation(a_prev, initraw, AF.Exp, bias=bias_sb)

        # ---- exp(x) staging ------------------------------------------------
        xhat = singles.tile([T, R * BL], F32)
        csz = 2048
        for k in range(R * BL // csz):
            rawc = raw.tile([T, csz], F32, name="rawc")
            nc.sync.dma_start(out=rawc, in_=xt_d.ap()[:, k * csz:(k + 1) * csz])
            nc.scalar.activation(xhat[:, k * csz:(k + 1) * csz], rawc, AF.Exp)

        # ---- the recurrence ------------------------------------------------
        nrms_sb = singles.tile([1, NRESC * BL], F32)
        pending = {}
        nrec = 0
        for r in range(1, rounds + 1):
            p = psum_p.tile([T, BL], F32, name="p")
            nc.tensor.matmul(p, lhsT=w_sb, rhs=a_prev, start=True, stop=True)
            if r == rounds:
                u_sb = singles.tile([T, BL], F32)
                nc.vector.tensor_copy(u_sb, p)
                nc.sync.dma_start(out=out_u_d.ap(), in_=u_sb)
                break
            a_new = apool.tile([T, BL], F32, name="a")
            nc.vector.tensor_mul(a_new, p, xhat[:, r * BL:(r + 1) * BL])
            if r in pending:
                a_res = apool.tile([T, BL], F32, name="a")
                nc.vector.tensor_mul(a_res, a_new, pending.pop(r))
                a_new = a_res
            if rescale and r in RESCALE_ROUNDS:
                nrm = psum_n.tile([1, BL], F32, name="nrm")
                nc.tensor.matmul(nrm, lhsT=onescol_sb, rhs=a_new, start=True, stop=True)
                rec = nrms_sb[:, nrec * BL:(nrec + 1) * BL]
                nc.vector.tensor_copy(rec, nrm)
                rcp = rpool.tile([1, BL], F32, name="rcp")
                nc.vector.reciprocal(rcp, rec)
                bc = psum_b.tile([T, BL], F32, name="bc")
                nc.tensor.matmul(bc, lhsT=onesrow_sb, rhs=rcp, start=True, stop=True)
                pending[r + DELAY] = bc
                nrec += 1
            a_prev = a_new
        assert (not rescale) or rounds < R or (nrec == NRESC and not pending)
        nc.sync.dma_start(out=out_a_d.ap(), in_=a_prev)
        nc.sync.dma_start(out=out_nrms_d.ap(), in_=nrms_sb)

        if numerator:
            # ---- numerator (one-hot emission + pair-count transition scores)
            # acc columns: [b*nchunk + k] emission partials, [NB*nchunk + b]
            # transition partials; the final ones-matmul sums over partitions,
            # the host sums the rest.
            nchunk = S // 128
            acc = singles.tile([128, NB * (nchunk + 1)], F32)
            for b in range(NB):
                if num_mode not in ("no_mm", "xn_only", "noxn", "p1", "p2"):
                    cp = psum_c.tile([128, T], F32, name="cp")
                for k in range(nchunk):
                    ecol = acc[:, b * nchunk + k:b * nchunk + k + 1]
                    if num_mode != "noxn":
                        xn = xn_pool.tile([128, T], F32, name="xn")
                        nc.sync.dma_start(
                            out=xn, in_=xnat_d.ap()[b, k * 128:(k + 1) * 128, :]
                        )
                    if num_mode == "xn_only":
                        nc.vector.reduce_sum(
                            out=ecol, in_=xn, axis=mybir.AxisListType.X
                        )
                        continue
                    if num_mode == "p1":
                        ohc = oh_pool.tile([128, T], F32, name="ohc")
                        nc.vector.tensor_scalar(
                            ohc, iota_sb,
                            tags_sb[:, k * NB + b:k * NB + b + 1], None,
                            OP.is_equal,
                        )
                        nc.vector.reduce_sum(
                            out=ecol, in_=ohc, axis=mybir.AxisListType.X
                        )
                        continue
                    if num_mode == "p2":
                        scr = scr_pool.tile([128, T], F32, name="scr")
                        nc.vector.tensor_tensor_reduce(
                            out=scr, in0=xn, in1=xn, scale=1.0, scalar=0.0,
                            op0=OP.mult, op1=OP.add, accum_out=ecol,
                        )
                        continue
                    e_in = iota_sb if num_mode == "noxn" else xn
                    ohc = oh_pool.tile([128, T], F32, name="ohc")
                    nc.vector.tensor_scalar(
                        ohc, iota_sb,
                        tags_sb[:, k * NB + b:k * NB + b + 1], None,
                        OP.is_equal,
                    )
                    scr = scr_pool.tile([128, T], F32, name="scr")
                    nc.vector.tensor_mul(scr, e_in, ohc)
                    nc.vector.reduce_sum(
                        out=ecol, in_=scr, axis=mybir.AxisListType.X
                    )
                    if num_mode not in ("no_mm", "noxn"):
                        ohn = oh_pool.tile([128, T], F32, name="ohn")
                        nc.vector.tensor_scalar(
                            ohn, iota_sb,
                            tagsn_sb[:, k * NB + b:k * NB + b + 1], None,
                            OP.is_equal,
                        )
                        nc.tensor.matmul(
                            cp, lhsT=ohc, rhs=ohn, start=(k == 0),
                            stop=(k == nchunk - 1),
                        )
                tcol = acc[:, NB * nchunk + b:NB * nchunk + b + 1]
                if num_mode not in ("no_mm", "xn_only", "noxn", "p1", "p2"):
                    scr2 = scr_pool.tile([128, T], F32, name="scr")
                    nc.vector.tensor_mul(scr2, cp, trans_sb)
                    nc.vector.reduce_sum(
                        out=tcol, in_=scr2, axis=mybir.AxisListType.X
                    )
                else:
                    nc.vector.reduce_sum(
                        out=tcol, in_=trans_sb, axis=mybir.AxisListType.X
                    )
            nump = psum_n.tile([1, NB * (nchunk + 1)], F32, name="nump")
            nc.tensor.matmul(nump, lhsT=onescol_sb, rhs=acc, start=True, stop=True)
            num_sb = singles.tile([1, NB * (nchunk + 1)], F32)
            nc.vector.tensor_copy(num_sb, nump)
            nc.sync.dma_start(out=out_num_d.ap(), in_=num_sb)

        if se:
            se_sb = singles.tile([2 * NB, 1], F32)
            nc.gpsimd.indirect_dma_start(
                out=se_sb,
                out_offset=None,
                in_=setable_d.ap(),
                in_offset=bass.IndirectOffsetOnAxis(ap=setags_sb[:, 0:1], axis=0),
            )
            nc.sync.dma_start(out=out_se_d.ap(), in_=se_sb)

    nc.compile()
    return nc


def prepare_in_maps(inputs):
    x = np.ascontiguousarray(np.asarray(inputs["inputs"], dtype=np.float32))
    tags = np.asarray(inputs["tags"]).astype(np.int32)
    trans = np.ascontiguousarray(np.asarray(inputs["transitions"], np.float32))
    start = np.asarray(inputs["start_transitions"], np.float32)
    end = np.asarray(inputs["end_transitions"], np.float32)

    iota_t = np.ascontiguousarray(
        np.broadcast_to(np.arange(T, dtype=np.float32), (128, T))
    )
    ones_col = np.ones((128, 1), np.float32)
    ones_row = np.ones((1, 128), np.float32)
    setable = np.concatenate([start, end]).reshape(2 * T, 1).astype(np.float32)
    transT = np.ascontiguousarray(trans.T)

    in_maps = []
    for c in range(8):
        fwd = c < 4
        g = c % 4
        xg = x[g * BL:(g + 1) * BL]            # [32, S, T]
        if fwd:
            xh = xg[:, :R, :]                  # s = 0..511
        else:
            xh = xg[:, R:, :][:, ::-1, :]      # s = 1023..512
        xt = np.ascontiguousarray(xh.transpose(2, 1, 0)).reshape(T, R * BL)

        tsh = tags[c * NB:(c + 1) * NB]        # [16, 1024]
        tags_sT = np.ascontiguousarray(
            tsh.T.reshape(S // 128, 128, NB).transpose(1, 0, 2)
        ).reshape(128, 128).astype(np.float32)
        tn = np.concatenate(
            [tsh[:, 1:], np.full((NB, 1), -1, np.int32)], axis=1
        )
        tagsn_sT = np.ascontiguousarray(
            tn.T.reshape(S // 128, 128, NB).transpose(1, 0, 2)
        ).reshape(128, 128).astype(np.float32)
        setags = np.concatenate(
            [tsh[:, 0], T + tsh[:, S - 1]]
        ).reshape(2 * NB, 1).astype(np.int32)

        in_maps.append({
            "xt": xt,
            "wraw": trans if fwd else transT,
            "bias": (start if fwd else end).reshape(T, 1).astype(np.float32),
            "transfull": trans,
            "tags_sT": tags_sT,
            "tagsn_sT": tagsn_sT,
            "setags": setags,
            "setable": setable,
            "iota_t": iota_t,
            "ones_col": ones_col,
            "ones_row": ones_row,
            "xnat": np.ascontiguousarray(x[c * NB:(c + 1) * NB]),
        })
    return in_maps


def assemble(results):
    """Host-side combine (the all-reduce): results = list of 8 per-core dicts."""
    den = np.zeros(B, np.float64)
    for g in range(4):
        u = results[g]["out_u"].astype(np.float64)          # [T, 32]
        a = results[4 + g]["out_a"].astype(np.float64)      # [T, 32]
        logs = np.zeros(BL, np.float64)
        for core in (g, 4 + g):
            nr = results[core]["out_nrms"].reshape(NRESC, BL).astype(np.float64)
            logs += np.log(nr).sum(axis=0)
        den[g * BL:(g + 1) * BL] = np.log((u * a).sum(axis=0)) + logs

    nchunk = S // 128
    num = np.zeros(B, np.float64)
    for c in range(8):
        se = results[c]["out_se"].reshape(2 * NB).astype(np.float64)
        nm = results[c]["out_num"].reshape(NB * (nchunk + 1)).astype(np.float64)
        emis = nm[:NB * nchunk].reshape(NB, nchunk).sum(axis=1)
        num[c * NB:(c + 1) * NB] = emis + nm[NB * nchunk:] + se[:NB] + se[NB:]
    return np.asarray((num - den).sum(), dtype=np.float32)


_CACHE = {}


def kernel(**inputs):
    if "nc" not in _CACHE:
        _CACHE["nc"] = build_program()
    nc = _CACHE["nc"]
    in_maps = prepare_in_maps(inputs)
    res = bass_utils.run_bass_kernel_spmd(nc, in_maps, core_ids=list(range(8)))
    return assemble(res.results)

